# revision 19
# baseline (speedup 1.0000x reference)
"""BatchBlur: depthwise 15x15 conv with per-sample kernels, reflection pad 7.

x: (32, 3, 512, 512) f32, kernel: (32, 15, 15) f32 -> out (32, 3, 512, 512) f32.

Strategy: pure data parallel over batch, 4 samples (12 channel-images) per
core on 8 cores. Host: reflection-pad x to (., 526, 526), store rows padded
to 528 with zero columns, cast to fp16, and build TRIPLE-band matrices
A[s, 42b + k, j, m] = kern[s, k-m, 3j+b] (0 <= k-m < 15, b in 0..2):
15 taps = 5 streams x 3 bands exactly.

Device: measured PE law on TRN2 - a matmul costs
  N * max(K_pad/256, M_pad/128) cycles   (pads to 32/64/128)
and consecutive matmuls on the same PE tile pay ~55 extra cycles of weight
-reload exposure unless >= 3 tiles run round-robin. So: M=28 output rows
per strip in 32-wide column tiles -> FOUR PE tiles round-robin (exposure
hidden), K = 3*42 = 126 bands (rows at column offsets 0/+1/+2, so one
stream covers taps 3j..3j+2 -> 5 streams for 15 taps):
  out[m, n] += sum_k A[k, j, m] * rhs[k, n + 3j]
Cost: 12 img x 19 strips x 5 streams = 1140 matmuls x 256 cycles ~ 122 us
(vs 8 streams x 11 strips x 311 cycles = 137 us for the dual-band M=50
layout - the 2-tile weight-reload exposure outweighs the extra DMA here).
Each strip's three bands come from ONE DMA per two strips: a 126-partition
transfer whose DRAM view is [band 3, row 42, strip 2, col] with band
stride 1 (the column shift) - an overlapping strided read. Loads and
stores round-robin across the Sync/GpSimd/Scalar queues to spread
descriptor generation. Strips: rows 0..503 in 18 28-row strips plus one
final strip at r0=484 (rows 484..525 = the padded end) storing rows
504..511. Eviction casts f32 PSUM to fp16 (DVE) and stores fp16; the host
casts back to f32 (adds ~4e-4 relative error).
"""
import os
import sys

for _p in ("/opt/trn_rl_repo", "/root/.axon_site/_ro/trn_rl_repo"):
    if _p not in sys.path and os.path.isdir(_p):
        sys.path.insert(0, _p)

import numpy as np

import concourse.bass as bass
import concourse.mybir as mybir
import concourse.tile as tile
from concourse import bacc
from concourse.bass_utils import run_bass_kernel_spmd

L = 15           # blur kernel size
P = L // 2       # reflection pad
B, C, H, W = 32, 3, 512, 512
N_CORES = 8
BS = B // N_CORES            # samples per core (4)
NIMG = BS * C                # channel images per core (12)
HP, WP = H + 2 * P, W + 2 * P  # 526
WPH = WP + 2                 # host row pitch: +2 zero cols so the shifted
                             # bands read defined data at their last columns
M = 28                       # output rows per strip (column-tile width 32)
KG = M + L - 1               # 42 rows per band
NB = 3                       # bands (column offsets 0, +1, +2)
K3 = NB * KG                 # 126 = contraction size
NS = 5                       # streams: taps {3j, 3j+1, 3j+2}
NFULL = 18                   # full strips (rows 0..503)
R0_LAST = H - M + 0          # 484: final strip, stores rows 504..511
LO_LAST = NFULL * M - R0_LAST  # 20: first stored row within final strip
N_WARMUP = 100               # dummy matmuls to release the HAM clock gate

F16 = mybir.dt.float16
F32 = mybir.dt.float32

_program_cache = None


def _build_program():
    nc = bacc.Bacc("TRN2", target_bir_lowering=False, debug=False)
    xp_d = nc.dram_tensor("xp", [NIMG, HP, WPH], F16,
                          kind="ExternalInput").ap()
    a_d = nc.dram_tensor("a", [BS, 128, NS, M], F16,
                         kind="ExternalInput").ap()
    out_d = nc.dram_tensor("out", [NIMG, H, W], F16,
                           kind="ExternalOutput").ap()

    with tile.TileContext(nc) as tc:
        with (
            tc.tile_pool(name="aconst", bufs=1) as apool,
            tc.tile_pool(name="warm", bufs=1) as wpool,
            tc.tile_pool(name="xin", bufs=6) as xpool,
            tc.tile_pool(name="oout", bufs=2) as opool,
            tc.tile_pool(name="psum", bufs=6, space="PSUM") as psum,
            tc.tile_pool(name="psumw", bufs=1, space="PSUM") as psumw,
        ):
            # HAM warm-up: a burst of matmuls on a zeroed scratch tile
            # releases the PE clock gate while the first input DMAs are in
            # flight.
            wsrc = wpool.tile([128, 64], mybir.dt.bfloat16)
            nc.gpsimd.memset(wsrc[:], 0.0)
            wacc = psumw.tile([64, 64], F32)
            for _ in range(N_WARMUP):
                nc.tensor.matmul(wacc[:], wsrc[:, :64], wsrc[:], start=True,
                                 stop=True)

            queues = [nc.sync, nc.gpsimd, nc.scalar]
            qctr = [0]

            def next_q():
                q = queues[qctr[0] % 3]
                qctr[0] += 1
                return q

            def load_strip(t, img, r0):
                # three simple 2-D DMAs, one per band: band b = the strip's
                # 42 rows at column offset +b, landing on partitions
                # 42b..42b+41. One queue per band spreads descriptor
                # generation across Sync/GpSimd/Scalar.
                base = (img * HP + r0) * WPH
                for b in range(NB):
                    queues[b].dma_start(
                        out=t[b * KG:(b + 1) * KG, :],
                        in_=bass.AP(xp_d.tensor, base + b,
                                    [[WPH, KG], [1, WP]]))

            # per-tile load units in consumption order: 18 strips then the
            # final overlap strip, for each of the tile's 3 images
            def issue_load(c, u):
                t, s = u // 19, u % 19
                img = 3 * c + t
                r0 = s * M if s < NFULL else R0_LAST
                xt = xpool.tile([128, WP], F16, tag=f"x{c}", name=f"x{c}")
                load_strip(xt, img, r0)
                return xt

            xt_cur = {}
            next_load = [0] * 4

            def ensure_loaded(u_need):
                # keep four strip loads in flight ahead of consumption
                for c in range(4):
                    while next_load[c] <= min(u_need + 4, 3 * 19 - 1):
                        un = next_load[c]
                        xt_cur[(c, un)] = issue_load(c, un)
                        next_load[c] += 1

            ensure_loaded(-1)

            a_t = [
                apool.tile([128, NS, M], F16, tag=f"a{s}", name=f"a{s}")
                for s in range(BS)
            ]
            for s in range(BS):
                nc.sync.dma_start(out=a_t[s][:], in_=a_d[s])

            # store batches per image: (first strip, count)
            sbatches = [(0, 4), (4, 4), (8, 4), (12, 4), (16, 2)]

            for t in range(C):
                for start, cnt in sbatches:
                    o_t = opool.tile([128, 4 * W], F16, tag="o", name="o")
                    for blk in range(cnt):
                        s = start + blk
                        u = t * 19 + s
                        ensure_loaded(u)
                        acc = psum.tile([128, W], F32, tag="ps", name="ps")
                        for j in range(NS):
                            for c in range(4):
                                nc.tensor.matmul(
                                    acc[32 * c:32 * c + M, :],
                                    a_t[c][0:K3, j, :],
                                    xt_cur[(c, u)][0:K3,
                                                   3 * j:3 * j + W],
                                    start=(j == 0),
                                    stop=(j == NS - 1),
                                    tile_position=(0, 32 * c),
                                )
                        nc.vector.tensor_copy(
                            out=o_t[:, blk * W:(blk + 1) * W], in_=acc[:])
                    for c in range(4):
                        img = 3 * c + t
                        dv = out_d[img, start * M:(start + cnt) * M,
                                   :].rearrange("(q p) c -> p q c", q=cnt)
                        sv = o_t[32 * c:32 * c + M, 0:cnt * W].rearrange(
                            "p (q c) -> p q c", c=W)
                        next_q().dma_start(out=dv, in_=sv)

                # final overlap strip: rows 484..511, store rows 504..511
                u = t * 19 + 18
                ensure_loaded(u)
                acc = psum.tile([128, W], F32, tag="ps", name="ps")
                for j in range(NS):
                    for c in range(4):
                        nc.tensor.matmul(
                            acc[32 * c:32 * c + M, :],
                            a_t[c][0:K3, j, :],
                            xt_cur[(c, u)][0:K3, 3 * j:3 * j + W],
                            start=(j == 0),
                            stop=(j == NS - 1),
                            tile_position=(0, 32 * c),
                        )
                o_s = opool.tile([128, W], F16, tag="o1", name="o1")
                nc.vector.tensor_copy(out=o_s[:], in_=acc[:])
                for c in range(4):
                    img = 3 * c + t
                    next_q().dma_start(
                        out=out_d[img, NFULL * M:H, :],
                        in_=o_s[32 * c + LO_LAST:32 * c + M, :])
    nc.compile()
    return nc


def prepare_in_maps(x: np.ndarray, kern: np.ndarray) -> list:
    # host-side reflection pad, fp16, rows padded to WPH with zero columns
    xpc = np.pad(x, ((0, 0), (0, 0), (P, P), (P, P)), mode="reflect")
    xp = np.zeros((B * C, HP, WPH), dtype=np.float16)
    xp[:, :, :WP] = xpc.reshape(B * C, HP, WP).astype(np.float16)

    # triple-band matrices: band b covers taps dx = 3j + b
    kern16 = kern.astype(np.float16)
    a_all = np.zeros((B, 128, NS, M), dtype=np.float16)
    m_idx = np.arange(M)
    for dx in range(L):
        j, b = dx // NB, dx % NB
        for dy in range(L):
            a_all[:, b * KG + m_idx + dy, j, m_idx] = kern16[:, dy, dx][:,
                                                                        None]
    return [
        {
            "xp": xp[c * NIMG:(c + 1) * NIMG],
            "a": a_all[c * BS:(c + 1) * BS],
        }
        for c in range(N_CORES)
    ]


def kernel(x: np.ndarray, kernel: np.ndarray) -> np.ndarray:
    global _program_cache
    x = np.asarray(x, dtype=np.float32)
    kern = np.asarray(kernel, dtype=np.float32)

    in_maps = prepare_in_maps(x, kern)
    if _program_cache is None:
        _program_cache = _build_program()
    nc = _program_cache

    res = run_bass_kernel_spmd(nc, in_maps, core_ids=list(range(N_CORES)))
    out = np.concatenate([r["out"] for r in res.results], axis=0)
    return out.reshape(B, C, H, W).astype(np.float32)


# revision 24
# speedup vs baseline: 2.2365x; 2.2365x over previous
"""BatchBlur: depthwise 15x15 conv with per-sample kernels, reflection pad 7.

x: (32, 3, 512, 512) f32, kernel: (32, 15, 15) f32 -> out (32, 3, 512, 512) f32.

Strategy: pure data parallel over batch, 4 samples per core on 8 cores.
Host: reflection-pad x to (., 526, 526), cast to fp16, and build dual-band
matrices A[s, k, j, m]: for k<64, A = kern[s, k-m, 2j]; for k>=64,
A = kern[s, k-64-m, 2j+1] (band condition 0 <= dy < 15).
Device: each rhs tile holds the strip rows TWICE — partitions 0:64 at
column offset 0 and partitions 64:128 at column offset 1 (two DMAs straight
from DRAM) — so a single accumulating matmul covers TWO horizontal taps
(dx=2j in the lower band, dx=2j+1 in the upper band) for 50 output rows:
  out[m, n] += sum_k A[k, j, m] * rhs[k, n + 2j]
Eight streams (j=0..7; j=7 contracts only the lower 64 partitions) replace
the fifteen per-tap matmuls. Two images run concurrently in the two
64-column halves of the PE array via tile_position=(0,0)/(0,64). fp16
operands keep the PE at 1 cycle/row (fp32 PSUM accumulation, ~3e-4 relative
error). Strips: output rows 0..499 in ten 50-row strips plus one final
strip at r0=462 (rows 462..525 = the exact end of the padded image) whose
store is sliced to rows 500..511.
"""
import os
import sys

for _p in ("/opt/trn_rl_repo", "/root/.axon_site/_ro/trn_rl_repo"):
    if _p not in sys.path and os.path.isdir(_p):
        sys.path.insert(0, _p)

import numpy as np

import concourse.bass as bass
import concourse.mybir as mybir
import concourse.tile as tile
from concourse import bacc
from concourse.bass_utils import run_bass_kernel_spmd

L = 15           # blur kernel size
P = L // 2       # reflection pad
B, C, H, W = 32, 3, 512, 512
N_CORES = 8
BS = B // N_CORES            # samples per core
NIMG = BS * C                # channel images per core
HP, WP = H + 2 * P, W + 2 * P  # 526
M_STRIP = 50                 # output rows per strip (dual-band: 2*(50+14)=128)
K_GRP = M_STRIP + L - 1      # 64 input rows per band group
N_DX = (L + 1) // 2          # 8 streams (two taps each; last is single)
R0S = [50 * s for s in range(10)] + [HP - K_GRP]  # last strip 462..525 exactly
N_WARMUP = 100               # dummy matmuls to release the HAM clock gate

F16 = mybir.dt.float16
F32 = mybir.dt.float32

_program_cache = None


def _build_program():
    nc = bacc.Bacc("TRN2", target_bir_lowering=False, debug=False)
    xp_d = nc.dram_tensor("xp", [NIMG, HP, WP], F16, kind="ExternalInput").ap()
    a_d = nc.dram_tensor("a", [BS, 128, N_DX, M_STRIP], F16,
                         kind="ExternalInput").ap()
    out_d = nc.dram_tensor("out", [NIMG, H, W], F32, kind="ExternalOutput").ap()

    def load_strip(t, img, r0):
        # lower band: rows at column offset 0 (Sync queue); upper band: same
        # rows at column offset 1 (GpSimd queue) => one matmul covers two
        # horizontal taps. Separate queues keep issue bandwidth in reserve.
        nc.sync.dma_start(out=t[0:K_GRP, :], in_=xp_d[img, r0:r0 + K_GRP, :])
        nc.gpsimd.dma_start(out=t[K_GRP:2 * K_GRP, 0:WP - 1],
                            in_=xp_d[img, r0:r0 + K_GRP, 1:WP])

    def load_strip2(t, img, r0):
        # double-strip load: one DMA per band group brings rows for strips
        # r0 and r0+50 (free-dim blocks 0:WP and WP:2*WP). The DRAM source
        # is an overlapping strided view (row stride WP, strip stride 50*WP)
        # — plain byte streams, legal for reads. Halves the DMA issue rate.
        base = (img * HP + r0) * WP
        nc.sync.dma_start(
            out=t[0:K_GRP, :].rearrange("p (q c) -> p q c", c=WP),
            in_=bass.AP(xp_d.tensor, base,
                        [[WP, K_GRP], [50 * WP, 2], [1, WP]]))
        nc.gpsimd.dma_start(
            out=t[K_GRP:2 * K_GRP, :].rearrange(
                "p (q c) -> p q c", c=WP)[:, :, 0:WP - 1],
            in_=bass.AP(xp_d.tensor, base + 1,
                        [[WP, K_GRP], [50 * WP, 2], [1, WP - 1]]))

    with tile.TileContext(nc) as tc:
        with (
            tc.tile_pool(name="aconst", bufs=1) as apool,
            tc.tile_pool(name="warm", bufs=1) as wpool,
            tc.tile_pool(name="xin", bufs=6) as xpool,
            tc.tile_pool(name="oout", bufs=4) as opool,
            tc.tile_pool(name="psum", bufs=6, space="PSUM") as psum,
            tc.tile_pool(name="psumw", bufs=1, space="PSUM") as psumw,
        ):
            # HAM warm-up: a burst of full-array matmuls on a zeroed scratch
            # tile releases the PE clock gate (col-tiled matmuls are invisible
            # to the HAM) while the first input DMAs are in flight.
            wsrc = wpool.tile([128, 64], mybir.dt.bfloat16)
            nc.gpsimd.memset(wsrc[:], 0.0)
            wacc = psumw.tile([64, 64], F32)
            for _ in range(N_WARMUP):
                nc.tensor.matmul(wacc[:], wsrc[:, :64], wsrc[:], start=True,
                                 stop=True)

            # The upper-band DMAs write columns 0..524 of each strip block
            # only; the last column of each block is read (x 0.0 weight) by
            # the j=7 stream, so it must be finite. Zero it once per slot.
            for slot in range(6):
                t = xpool.tile([128, 2 * WP], F16, tag="xp2", name="xz2")
                nc.gpsimd.memset(t[K_GRP:2 * K_GRP, WP - 1:WP], 0.0)
                nc.gpsimd.memset(t[K_GRP:2 * K_GRP, 2 * WP - 1:2 * WP], 0.0)
            for slot in range(6):
                t = xpool.tile([128, WP], F16, tag="xp_t", name="xz1")
                nc.gpsimd.memset(t[K_GRP:2 * K_GRP, WP - 1:WP], 0.0)

            # first double-strip's image rows: issued before the A load so
            # the DMA queues deliver the first matmuls' dependencies earliest
            xp_first = []
            for img in range(2):
                t = xpool.tile([128, 2 * WP], F16, tag="xp2", name=f"xpf{img}")
                load_strip2(t, img, 0)
                xp_first.append(t)

            # per-sample dual-band matrices: separate tiles => separate
            # dependency tracking; later samples load lazily
            a_t = [
                apool.tile([128, N_DX, M_STRIP], F16, tag=f"a{s}",
                           name=f"a{s}")
                for s in range(BS)
            ]
            nc.sync.dma_start(out=a_t[0][:], in_=a_d[0])

            a_loaded = 0
            for pair in range(NIMG // 2):
                img_a, img_b = 2 * pair, 2 * pair + 1
                smp_a, smp_b = img_a // C, img_b // C
                for s_need in ((2 * pair + 2) // C, (2 * pair + 3) // C):
                    if s_need < BS and s_need > a_loaded:
                        nc.sync.dma_start(out=a_t[s_need][:], in_=a_d[s_need])
                        a_loaded = s_need

                # five double-strip units (rows 0..499) + one single overlap
                # strip at r0=462 storing rows 500..511
                for du in range(6):
                    if du < 5:
                        r0 = 100 * du
                        if pair == 0 and du == 0:
                            xa, xb = xp_first
                        else:
                            xa = xpool.tile([128, 2 * WP], F16, tag="xp2",
                                            name="xa")
                            load_strip2(xa, img_a, r0)
                            xb = xpool.tile([128, 2 * WP], F16, tag="xp2",
                                            name="xb")
                            load_strip2(xb, img_b, r0)
                        o_t = opool.tile([128, 2 * W], F32)
                        for sub in range(2):
                            base = sub * WP
                            acc = psum.tile([128, W], F32)
                            # all 8 streams use K=128 (j=7's upper band is
                            # zero weights) — a K=64 stream would switch the
                            # PE tiling mode and pay a drain twice per strip
                            for j in range(N_DX):
                                nc.tensor.matmul(
                                    acc[0:M_STRIP],
                                    a_t[smp_a][:, j, :],
                                    xa[:, base + 2 * j:base + 2 * j + W],
                                    start=(j == 0),
                                    stop=(j == N_DX - 1),
                                    tile_position=(0, 0),
                                )
                                nc.tensor.matmul(
                                    acc[64:64 + M_STRIP],
                                    a_t[smp_b][:, j, :],
                                    xb[:, base + 2 * j:base + 2 * j + W],
                                    start=(j == 0),
                                    stop=(j == N_DX - 1),
                                    tile_position=(0, 64),
                                )
                            nc.vector.tensor_copy(
                                out=o_t[:, sub * W:(sub + 1) * W],
                                in_=acc[:])
                        # one store per image covers both strips (100
                        # contiguous output rows; non-overlapping views)
                        dva = out_d[img_a, r0:r0 + 2 * M_STRIP, :].rearrange(
                            "(q p) c -> p q c", q=2)
                        dvb = out_d[img_b, r0:r0 + 2 * M_STRIP, :].rearrange(
                            "(q p) c -> p q c", q=2)
                        sva = o_t[0:M_STRIP, :].rearrange(
                            "p (q c) -> p q c", c=W)
                        svb = o_t[64:64 + M_STRIP, :].rearrange(
                            "p (q c) -> p q c", c=W)
                        nc.scalar.dma_start(out=dva, in_=sva)
                        nc.scalar.dma_start(out=dvb, in_=svb)
                    else:
                        r0 = R0S[-1]  # 462
                        lo = 10 * M_STRIP - r0  # store rows 500..511 only
                        xa = xpool.tile([128, WP], F16, tag="xp_t", name="xa1")
                        load_strip(xa, img_a, r0)
                        xb = xpool.tile([128, WP], F16, tag="xp_t", name="xb1")
                        load_strip(xb, img_b, r0)
                        acc = psum.tile([128, W], F32)
                        for j in range(N_DX):
                            nc.tensor.matmul(
                                acc[0:M_STRIP], a_t[smp_a][:, j, :],
                                xa[:, 2 * j:2 * j + W], start=(j == 0),
                                stop=(j == N_DX - 1), tile_position=(0, 0))
                            nc.tensor.matmul(
                                acc[64:64 + M_STRIP], a_t[smp_b][:, j, :],
                                xb[:, 2 * j:2 * j + W], start=(j == 0),
                                stop=(j == N_DX - 1), tile_position=(0, 64))
                        o_s = opool.tile([128, W], F32, tag="o1", name="o1")
                        nc.vector.tensor_copy(out=o_s[:], in_=acc[:])
                        nc.scalar.dma_start(
                            out=out_d[img_a, r0 + lo:r0 + M_STRIP, :],
                            in_=o_s[lo:M_STRIP])
                        nc.scalar.dma_start(
                            out=out_d[img_b, r0 + lo:r0 + M_STRIP, :],
                            in_=o_s[64 + lo:64 + M_STRIP])
    nc.compile()
    return nc


def prepare_in_maps(x: np.ndarray, kern: np.ndarray) -> list:
    # host-side reflection pad, cast to fp16 for half the DMA bytes
    xp = np.pad(x, ((0, 0), (0, 0), (P, P), (P, P)), mode="reflect")
    xp = np.ascontiguousarray(
        xp.reshape(B * C, HP, WP).astype(np.float16))

    # dual-band matrices: lower band = even taps, upper band = odd taps
    kern16 = kern.astype(np.float16)
    a_all = np.zeros((B, 128, N_DX, M_STRIP), dtype=np.float16)
    m_idx = np.arange(M_STRIP)
    for dy in range(L):
        a_all[:, m_idx + dy, :, m_idx] = kern16[:, dy, 0::2]
        a_all[:, K_GRP + m_idx + dy, :L // 2, m_idx] = kern16[:, dy, 1::2]

    return [
        {
            "xp": xp[c * NIMG:(c + 1) * NIMG],
            "a": a_all[c * BS:(c + 1) * BS],
        }
        for c in range(N_CORES)
    ]


def kernel(x: np.ndarray, kernel: np.ndarray) -> np.ndarray:
    global _program_cache
    x = np.asarray(x, dtype=np.float32)
    kern = np.asarray(kernel, dtype=np.float32)

    in_maps = prepare_in_maps(x, kern)
    if _program_cache is None:
        _program_cache = _build_program()
    nc = _program_cache

    res = run_bass_kernel_spmd(nc, in_maps, core_ids=list(range(N_CORES)))
    out = np.concatenate([r["out"] for r in res.results], axis=0)
    return out.reshape(B, C, H, W)


# revision 28
# speedup vs baseline: 2.2749x; 1.0172x over previous
"""BatchBlur: depthwise 15x15 conv with per-sample kernels, reflection pad 7.

x: (32, 3, 512, 512) f32, kernel: (32, 15, 15) f32 -> out (32, 3, 512, 512) f32.

Strategy: pure data parallel over batch, 4 samples per core on 8 cores.
Host: reflection-pad x to (., 526, 526), cast to fp16, and build dual-band
matrices A[s, k, j, m]: for k<64, A = kern[s, k-m, 2j]; for k>=64,
A = kern[s, k-64-m, 2j+1] (band condition 0 <= dy < 15).
Device: each rhs tile holds the strip rows TWICE — partitions 0:64 at
column offset 0 and partitions 64:128 at column offset 1 (two DMAs straight
from DRAM) — so a single accumulating matmul covers TWO horizontal taps
(dx=2j in the lower band, dx=2j+1 in the upper band) for 50 output rows:
  out[m, n] += sum_k A[k, j, m] * rhs[k, n + 2j]
Eight streams (j=0..7; j=7 contracts only the lower 64 partitions) replace
the fifteen per-tap matmuls. Two images run concurrently in the two
64-column halves of the PE array via tile_position=(0,0)/(0,64). fp16
operands keep the PE at 1 cycle/row (fp32 PSUM accumulation, ~3e-4 relative
error). Strips: output rows 0..499 in ten 50-row strips plus one final
strip at r0=462 (rows 462..525 = the exact end of the padded image) whose
store is sliced to rows 500..511.
"""
import os
import sys

for _p in ("/opt/trn_rl_repo", "/root/.axon_site/_ro/trn_rl_repo"):
    if _p not in sys.path and os.path.isdir(_p):
        sys.path.insert(0, _p)

import numpy as np

import concourse.bass as bass
import concourse.mybir as mybir
import concourse.tile as tile
from concourse import bacc
from concourse.bass_utils import run_bass_kernel_spmd

L = 15           # blur kernel size
P = L // 2       # reflection pad
B, C, H, W = 32, 3, 512, 512
N_CORES = 8
BS = B // N_CORES            # samples per core
NIMG = BS * C                # channel images per core
HP, WP = H + 2 * P, W + 2 * P  # 526
M_STRIP = 50                 # output rows per strip (dual-band: 2*(50+14)=128)
K_GRP = M_STRIP + L - 1      # 64 input rows per band group
N_DX = (L + 1) // 2          # 8 streams (two taps each; last is single)
R0S = [50 * s for s in range(10)] + [HP - K_GRP]  # last strip 462..525 exactly
N_WARMUP = 100               # dummy matmuls to release the HAM clock gate

F16 = mybir.dt.float16
F32 = mybir.dt.float32

_program_cache = None


def _build_program():
    nc = bacc.Bacc("TRN2", target_bir_lowering=False, debug=False)
    xp_d = nc.dram_tensor("xp", [NIMG, HP, WP], F16, kind="ExternalInput").ap()
    a_d = nc.dram_tensor("a", [BS, 128, N_DX, M_STRIP], F16,
                         kind="ExternalInput").ap()
    out_d = nc.dram_tensor("out", [NIMG, H, W], F16, kind="ExternalOutput").ap()

    def load_strip(t, img, r0):
        # lower band: rows at column offset 0 (Sync queue); upper band: same
        # rows at column offset 1 (GpSimd queue) => one matmul covers two
        # horizontal taps. Separate queues keep issue bandwidth in reserve.
        nc.sync.dma_start(out=t[0:K_GRP, :], in_=xp_d[img, r0:r0 + K_GRP, :])
        nc.gpsimd.dma_start(out=t[K_GRP:2 * K_GRP, 0:WP - 1],
                            in_=xp_d[img, r0:r0 + K_GRP, 1:WP])

    def load_strip2(t, img, r0):
        # double-strip load: one DMA per band group brings rows for strips
        # r0 and r0+50 (free-dim blocks 0:WP and WP:2*WP). The DRAM source
        # is an overlapping strided view (row stride WP, strip stride 50*WP)
        # — plain byte streams, legal for reads. Halves the DMA issue rate.
        base = (img * HP + r0) * WP
        nc.sync.dma_start(
            out=t[0:K_GRP, :].rearrange("p (q c) -> p q c", c=WP),
            in_=bass.AP(xp_d.tensor, base,
                        [[WP, K_GRP], [50 * WP, 2], [1, WP]]))
        nc.gpsimd.dma_start(
            out=t[K_GRP:2 * K_GRP, :].rearrange(
                "p (q c) -> p q c", c=WP)[:, :, 0:WP - 1],
            in_=bass.AP(xp_d.tensor, base + 1,
                        [[WP, K_GRP], [50 * WP, 2], [1, WP - 1]]))

    with tile.TileContext(nc) as tc:
        with (
            tc.tile_pool(name="aconst", bufs=1) as apool,
            tc.tile_pool(name="warm", bufs=1) as wpool,
            tc.tile_pool(name="xin", bufs=6) as xpool,
            tc.tile_pool(name="oout", bufs=4) as opool,
            tc.tile_pool(name="psum", bufs=6, space="PSUM") as psum,
            tc.tile_pool(name="psumw", bufs=1, space="PSUM") as psumw,
        ):
            # HAM warm-up: a burst of full-array matmuls on a zeroed scratch
            # tile releases the PE clock gate (col-tiled matmuls are invisible
            # to the HAM) while the first input DMAs are in flight.
            wsrc = wpool.tile([128, 64], mybir.dt.bfloat16)
            nc.gpsimd.memset(wsrc[:], 0.0)
            wacc = psumw.tile([64, 64], F32)
            for _ in range(N_WARMUP):
                nc.tensor.matmul(wacc[:], wsrc[:, :64], wsrc[:], start=True,
                                 stop=True)

            # The upper-band DMAs write columns 0..524 of each strip block
            # only; the last column of each block is read (x 0.0 weight) by
            # the j=7 stream, so it must be finite. Zero it once per slot.
            for slot in range(6):
                t = xpool.tile([128, 2 * WP], F16, tag="xp2", name="xz2")
                nc.gpsimd.memset(t[K_GRP:2 * K_GRP, WP - 1:WP], 0.0)
                nc.gpsimd.memset(t[K_GRP:2 * K_GRP, 2 * WP - 1:2 * WP], 0.0)
            for slot in range(6):
                t = xpool.tile([128, WP], F16, tag="xp_t", name="xz1")
                nc.gpsimd.memset(t[K_GRP:2 * K_GRP, WP - 1:WP], 0.0)

            # first double-strip's image rows: issued before the A load so
            # the DMA queues deliver the first matmuls' dependencies earliest
            xp_first = []
            for img in range(2):
                t = xpool.tile([128, 2 * WP], F16, tag="xp2", name=f"xpf{img}")
                load_strip2(t, img, 0)
                xp_first.append(t)

            # per-sample dual-band matrices: separate tiles => separate
            # dependency tracking; later samples load lazily
            a_t = [
                apool.tile([128, N_DX, M_STRIP], F16, tag=f"a{s}",
                           name=f"a{s}")
                for s in range(BS)
            ]
            nc.sync.dma_start(out=a_t[0][:], in_=a_d[0])

            a_loaded = 0
            for pair in range(NIMG // 2):
                img_a, img_b = 2 * pair, 2 * pair + 1
                smp_a, smp_b = img_a // C, img_b // C
                for s_need in ((2 * pair + 2) // C, (2 * pair + 3) // C):
                    if s_need < BS and s_need > a_loaded:
                        nc.sync.dma_start(out=a_t[s_need][:], in_=a_d[s_need])
                        a_loaded = s_need

                # five double-strip units (rows 0..499) + one single overlap
                # strip at r0=462 storing rows 500..511
                for du in range(6):
                    if du < 5:
                        r0 = 100 * du
                        if pair == 0 and du == 0:
                            xa, xb = xp_first
                        else:
                            xa = xpool.tile([128, 2 * WP], F16, tag="xp2",
                                            name="xa")
                            load_strip2(xa, img_a, r0)
                            xb = xpool.tile([128, 2 * WP], F16, tag="xp2",
                                            name="xb")
                            load_strip2(xb, img_b, r0)
                        # evict as fp16 (DVE casts f32 PSUM) — halves store
                        # bytes; the host casts back to f32 (~2e-4 rel err)
                        o_t = opool.tile([128, 2 * W], F16)
                        for sub in range(2):
                            base = sub * WP
                            acc = psum.tile([128, W], F32)
                            # all 8 streams use K=128 (j=7's upper band is
                            # zero weights) — a K=64 stream would switch the
                            # PE tiling mode and pay a drain twice per strip
                            for j in range(N_DX):
                                nc.tensor.matmul(
                                    acc[0:M_STRIP],
                                    a_t[smp_a][:, j, :],
                                    xa[:, base + 2 * j:base + 2 * j + W],
                                    start=(j == 0),
                                    stop=(j == N_DX - 1),
                                    tile_position=(0, 0),
                                )
                                nc.tensor.matmul(
                                    acc[64:64 + M_STRIP],
                                    a_t[smp_b][:, j, :],
                                    xb[:, base + 2 * j:base + 2 * j + W],
                                    start=(j == 0),
                                    stop=(j == N_DX - 1),
                                    tile_position=(0, 64),
                                )
                            nc.vector.tensor_copy(
                                out=o_t[:, sub * W:(sub + 1) * W],
                                in_=acc[:])
                        # one store per image covers both strips (100
                        # contiguous output rows; non-overlapping views)
                        dva = out_d[img_a, r0:r0 + 2 * M_STRIP, :].rearrange(
                            "(q p) c -> p q c", q=2)
                        dvb = out_d[img_b, r0:r0 + 2 * M_STRIP, :].rearrange(
                            "(q p) c -> p q c", q=2)
                        sva = o_t[0:M_STRIP, :].rearrange(
                            "p (q c) -> p q c", c=W)
                        svb = o_t[64:64 + M_STRIP, :].rearrange(
                            "p (q c) -> p q c", c=W)
                        nc.scalar.dma_start(out=dva, in_=sva)
                        nc.scalar.dma_start(out=dvb, in_=svb)
                    else:
                        r0 = R0S[-1]  # 462
                        lo = 10 * M_STRIP - r0  # store rows 500..511 only
                        xa = xpool.tile([128, WP], F16, tag="xp_t", name="xa1")
                        load_strip(xa, img_a, r0)
                        xb = xpool.tile([128, WP], F16, tag="xp_t", name="xb1")
                        load_strip(xb, img_b, r0)
                        acc = psum.tile([128, W], F32)
                        for j in range(N_DX):
                            nc.tensor.matmul(
                                acc[0:M_STRIP], a_t[smp_a][:, j, :],
                                xa[:, 2 * j:2 * j + W], start=(j == 0),
                                stop=(j == N_DX - 1), tile_position=(0, 0))
                            nc.tensor.matmul(
                                acc[64:64 + M_STRIP], a_t[smp_b][:, j, :],
                                xb[:, 2 * j:2 * j + W], start=(j == 0),
                                stop=(j == N_DX - 1), tile_position=(0, 64))
                        o_s = opool.tile([128, W], F16, tag="o1", name="o1")
                        nc.vector.tensor_copy(out=o_s[:], in_=acc[:])
                        nc.scalar.dma_start(
                            out=out_d[img_a, r0 + lo:r0 + M_STRIP, :],
                            in_=o_s[lo:M_STRIP])
                        nc.scalar.dma_start(
                            out=out_d[img_b, r0 + lo:r0 + M_STRIP, :],
                            in_=o_s[64 + lo:64 + M_STRIP])
    nc.compile()
    return nc


def prepare_in_maps(x: np.ndarray, kern: np.ndarray) -> list:
    # host-side reflection pad, cast to fp16 for half the DMA bytes
    xp = np.pad(x, ((0, 0), (0, 0), (P, P), (P, P)), mode="reflect")
    xp = np.ascontiguousarray(
        xp.reshape(B * C, HP, WP).astype(np.float16))

    # dual-band matrices: lower band = even taps, upper band = odd taps
    kern16 = kern.astype(np.float16)
    a_all = np.zeros((B, 128, N_DX, M_STRIP), dtype=np.float16)
    m_idx = np.arange(M_STRIP)
    for dy in range(L):
        a_all[:, m_idx + dy, :, m_idx] = kern16[:, dy, 0::2]
        a_all[:, K_GRP + m_idx + dy, :L // 2, m_idx] = kern16[:, dy, 1::2]

    return [
        {
            "xp": xp[c * NIMG:(c + 1) * NIMG],
            "a": a_all[c * BS:(c + 1) * BS],
        }
        for c in range(N_CORES)
    ]


def kernel(x: np.ndarray, kernel: np.ndarray) -> np.ndarray:
    global _program_cache
    x = np.asarray(x, dtype=np.float32)
    kern = np.asarray(kernel, dtype=np.float32)

    in_maps = prepare_in_maps(x, kern)
    if _program_cache is None:
        _program_cache = _build_program()
    nc = _program_cache

    res = run_bass_kernel_spmd(nc, in_maps, core_ids=list(range(N_CORES)))
    out = np.concatenate([r["out"] for r in res.results], axis=0)
    return out.reshape(B, C, H, W).astype(np.float32)


# revision 29
# speedup vs baseline: 2.2751x; 1.0001x over previous
"""BatchBlur: depthwise 15x15 conv with per-sample kernels, reflection pad 7.

x: (32, 3, 512, 512) f32, kernel: (32, 15, 15) f32 -> out (32, 3, 512, 512) f32.

Strategy: pure data parallel over batch, 4 samples per core on 8 cores.
Host: reflection-pad x to (., 526, 526), cast to fp16, and build dual-band
matrices A[s, k, j, m]: for k<64, A = kern[s, k-m, 2j]; for k>=64,
A = kern[s, k-64-m, 2j+1] (band condition 0 <= dy < 15).
Device: each rhs tile holds the strip rows TWICE — partitions 0:64 at
column offset 0 and partitions 64:128 at column offset 1 (two DMAs straight
from DRAM) — so a single accumulating matmul covers TWO horizontal taps
(dx=2j in the lower band, dx=2j+1 in the upper band) for 50 output rows:
  out[m, n] += sum_k A[k, j, m] * rhs[k, n + 2j]
Eight streams (j=0..7; j=7 contracts only the lower 64 partitions) replace
the fifteen per-tap matmuls. Two images alternate in the two 64-column
halves of the PE array via tile_position=(0,0)/(0,64). fp16 operands with
fp32 PSUM accumulation; eviction casts to fp16 and stores fp16 (halving
store bytes), the host casts back to f32 (~6e-4 relative error total).
Strips: output rows 0..499 in ten 50-row strips plus one final strip at
r0=462 (rows 462..525 = the exact end of the padded image) whose store is
sliced to rows 500..511.
"""
import os
import sys

for _p in ("/opt/trn_rl_repo", "/root/.axon_site/_ro/trn_rl_repo"):
    if _p not in sys.path and os.path.isdir(_p):
        sys.path.insert(0, _p)

import numpy as np

import concourse.bass as bass
import concourse.mybir as mybir
import concourse.tile as tile
from concourse import bacc
from concourse.bass_utils import run_bass_kernel_spmd

L = 15           # blur kernel size
P = L // 2       # reflection pad
B, C, H, W = 32, 3, 512, 512
N_CORES = 8
BS = B // N_CORES            # samples per core
NIMG = BS * C                # channel images per core
HP, WP = H + 2 * P, W + 2 * P  # 526
M_STRIP = 50                 # output rows per strip (dual-band: 2*(50+14)=128)
K_GRP = M_STRIP + L - 1      # 64 input rows per band group
N_DX = (L + 1) // 2          # 8 streams (two taps each; last is single)
R0S = [50 * s for s in range(10)] + [HP - K_GRP]  # last strip 462..525 exactly
N_WARMUP = 100               # dummy matmuls to release the HAM clock gate

F16 = mybir.dt.float16
F32 = mybir.dt.float32

_program_cache = None


def _build_program():
    nc = bacc.Bacc("TRN2", target_bir_lowering=False, debug=False)
    xp_d = nc.dram_tensor("xp", [NIMG, HP, WP], F16, kind="ExternalInput").ap()
    a_d = nc.dram_tensor("a", [BS, 128, N_DX, M_STRIP], F16,
                         kind="ExternalInput").ap()
    out_d = nc.dram_tensor("out", [NIMG, H, W], F16, kind="ExternalOutput").ap()

    def load_strip(t, img, r0):
        # lower band: rows at column offset 0 (Sync queue); upper band: same
        # rows at column offset 1 (GpSimd queue) => one matmul covers two
        # horizontal taps. Separate queues keep issue bandwidth in reserve.
        nc.sync.dma_start(out=t[0:K_GRP, :], in_=xp_d[img, r0:r0 + K_GRP, :])
        nc.gpsimd.dma_start(out=t[K_GRP:2 * K_GRP, 0:WP - 1],
                            in_=xp_d[img, r0:r0 + K_GRP, 1:WP])

    def load_strip2(t, img, r0):
        # double-strip load: one DMA per band group brings rows for strips
        # r0 and r0+50 (free-dim blocks 0:WP and WP:2*WP). The DRAM source
        # is an overlapping strided view (row stride WP, strip stride 50*WP)
        # — plain byte streams, legal for reads. Halves the DMA issue rate.
        base = (img * HP + r0) * WP
        nc.sync.dma_start(
            out=t[0:K_GRP, :].rearrange("p (q c) -> p q c", c=WP),
            in_=bass.AP(xp_d.tensor, base,
                        [[WP, K_GRP], [50 * WP, 2], [1, WP]]))
        nc.gpsimd.dma_start(
            out=t[K_GRP:2 * K_GRP, :].rearrange(
                "p (q c) -> p q c", c=WP)[:, :, 0:WP - 1],
            in_=bass.AP(xp_d.tensor, base + 1,
                        [[WP, K_GRP], [50 * WP, 2], [1, WP - 1]]))

    with tile.TileContext(nc) as tc:
        with (
            tc.tile_pool(name="aconst", bufs=1) as apool,
            tc.tile_pool(name="warm", bufs=1) as wpool,
            tc.tile_pool(name="xin", bufs=6) as xpool,
            tc.tile_pool(name="oout", bufs=4) as opool,
            tc.tile_pool(name="psum", bufs=6, space="PSUM") as psum,
            tc.tile_pool(name="psumw", bufs=1, space="PSUM") as psumw,
        ):
            # HAM warm-up: a burst of full-array matmuls on a zeroed scratch
            # tile releases the PE clock gate (col-tiled matmuls are invisible
            # to the HAM) while the first input DMAs are in flight.
            wsrc = wpool.tile([128, 64], mybir.dt.bfloat16)
            nc.gpsimd.memset(wsrc[:], 0.0)
            wacc = psumw.tile([64, 64], F32)
            for _ in range(N_WARMUP):
                nc.tensor.matmul(wacc[:], wsrc[:, :64], wsrc[:], start=True,
                                 stop=True)

            # The upper-band DMAs write columns 0..524 of each strip block
            # only; the last column of each block is read (x 0.0 weight) by
            # the j=7 stream, so it must be finite. Zero it once per slot.
            for slot in range(6):
                t = xpool.tile([128, 2 * WP], F16, tag="xp2", name="xz2")
                nc.gpsimd.memset(t[K_GRP:2 * K_GRP, WP - 1:WP], 0.0)
                nc.gpsimd.memset(t[K_GRP:2 * K_GRP, 2 * WP - 1:2 * WP], 0.0)
            for slot in range(6):
                t = xpool.tile([128, WP], F16, tag="xp_t", name="xz1")
                nc.gpsimd.memset(t[K_GRP:2 * K_GRP, WP - 1:WP], 0.0)

            # first double-strip's image rows: issued before the A load so
            # the DMA queues deliver the first matmuls' dependencies earliest
            xp_first = []
            for img in range(2):
                t = xpool.tile([128, 2 * WP], F16, tag="xp2", name=f"xpf{img}")
                load_strip2(t, img, 0)
                xp_first.append(t)

            # per-sample dual-band matrices: separate tiles => separate
            # dependency tracking; later samples load lazily
            a_t = [
                apool.tile([128, N_DX, M_STRIP], F16, tag=f"a{s}",
                           name=f"a{s}")
                for s in range(BS)
            ]
            nc.sync.dma_start(out=a_t[0][:], in_=a_d[0])

            a_loaded = 0
            for pair in range(NIMG // 2):
                img_a, img_b = 2 * pair, 2 * pair + 1
                smp_a, smp_b = img_a // C, img_b // C
                for s_need in ((2 * pair + 2) // C, (2 * pair + 3) // C):
                    if s_need < BS and s_need > a_loaded:
                        nc.sync.dma_start(out=a_t[s_need][:], in_=a_d[s_need])
                        a_loaded = s_need

                # five double-strip units (rows 0..499) + one single overlap
                # strip at r0=462 storing rows 500..511
                for du in range(6):
                    if du < 5:
                        r0 = 100 * du
                        if pair == 0 and du == 0:
                            xa, xb = xp_first
                        else:
                            xa = xpool.tile([128, 2 * WP], F16, tag="xp2",
                                            name="xa")
                            load_strip2(xa, img_a, r0)
                            xb = xpool.tile([128, 2 * WP], F16, tag="xp2",
                                            name="xb")
                            load_strip2(xb, img_b, r0)
                        # evict as fp16 (DVE casts f32 PSUM) — halves store
                        # bytes; the host casts back to f32 (~2e-4 rel err)
                        o_t = opool.tile([128, 2 * W], F16)
                        for sub in range(2):
                            base = sub * WP
                            acc = psum.tile([128, W], F32)
                            # all 8 streams use K=128 (j=7's upper band is
                            # zero weights) — a K=64 stream would switch the
                            # PE tiling mode and pay a drain twice per strip
                            for j in range(N_DX):
                                nc.tensor.matmul(
                                    acc[0:M_STRIP],
                                    a_t[smp_a][:, j, :],
                                    xa[:, base + 2 * j:base + 2 * j + W],
                                    start=(j == 0),
                                    stop=(j == N_DX - 1),
                                    tile_position=(0, 0),
                                )
                                nc.tensor.matmul(
                                    acc[64:64 + M_STRIP],
                                    a_t[smp_b][:, j, :],
                                    xb[:, base + 2 * j:base + 2 * j + W],
                                    start=(j == 0),
                                    stop=(j == N_DX - 1),
                                    tile_position=(0, 64),
                                )
                            nc.vector.tensor_copy(
                                out=o_t[:, sub * W:(sub + 1) * W],
                                in_=acc[:])
                        # one store per image covers both strips (100
                        # contiguous output rows; non-overlapping views)
                        dva = out_d[img_a, r0:r0 + 2 * M_STRIP, :].rearrange(
                            "(q p) c -> p q c", q=2)
                        dvb = out_d[img_b, r0:r0 + 2 * M_STRIP, :].rearrange(
                            "(q p) c -> p q c", q=2)
                        sva = o_t[0:M_STRIP, :].rearrange(
                            "p (q c) -> p q c", c=W)
                        svb = o_t[64:64 + M_STRIP, :].rearrange(
                            "p (q c) -> p q c", c=W)
                        nc.scalar.dma_start(out=dva, in_=sva)
                        nc.scalar.dma_start(out=dvb, in_=svb)
                    else:
                        r0 = R0S[-1]  # 462
                        lo = 10 * M_STRIP - r0  # store rows 500..511 only
                        xa = xpool.tile([128, WP], F16, tag="xp_t", name="xa1")
                        load_strip(xa, img_a, r0)
                        xb = xpool.tile([128, WP], F16, tag="xp_t", name="xb1")
                        load_strip(xb, img_b, r0)
                        acc = psum.tile([128, W], F32)
                        for j in range(N_DX):
                            nc.tensor.matmul(
                                acc[0:M_STRIP], a_t[smp_a][:, j, :],
                                xa[:, 2 * j:2 * j + W], start=(j == 0),
                                stop=(j == N_DX - 1), tile_position=(0, 0))
                            nc.tensor.matmul(
                                acc[64:64 + M_STRIP], a_t[smp_b][:, j, :],
                                xb[:, 2 * j:2 * j + W], start=(j == 0),
                                stop=(j == N_DX - 1), tile_position=(0, 64))
                        o_s = opool.tile([128, W], F16, tag="o1", name="o1")
                        nc.vector.tensor_copy(out=o_s[:], in_=acc[:])
                        nc.scalar.dma_start(
                            out=out_d[img_a, r0 + lo:r0 + M_STRIP, :],
                            in_=o_s[lo:M_STRIP])
                        nc.scalar.dma_start(
                            out=out_d[img_b, r0 + lo:r0 + M_STRIP, :],
                            in_=o_s[64 + lo:64 + M_STRIP])
    nc.compile()
    return nc


def prepare_in_maps(x: np.ndarray, kern: np.ndarray) -> list:
    # host-side reflection pad, cast to fp16 for half the DMA bytes
    xp = np.pad(x, ((0, 0), (0, 0), (P, P), (P, P)), mode="reflect")
    xp = np.ascontiguousarray(
        xp.reshape(B * C, HP, WP).astype(np.float16))

    # dual-band matrices: lower band = even taps, upper band = odd taps
    kern16 = kern.astype(np.float16)
    a_all = np.zeros((B, 128, N_DX, M_STRIP), dtype=np.float16)
    m_idx = np.arange(M_STRIP)
    for dy in range(L):
        a_all[:, m_idx + dy, :, m_idx] = kern16[:, dy, 0::2]
        a_all[:, K_GRP + m_idx + dy, :L // 2, m_idx] = kern16[:, dy, 1::2]

    return [
        {
            "xp": xp[c * NIMG:(c + 1) * NIMG],
            "a": a_all[c * BS:(c + 1) * BS],
        }
        for c in range(N_CORES)
    ]


def kernel(x: np.ndarray, kernel: np.ndarray) -> np.ndarray:
    global _program_cache
    x = np.asarray(x, dtype=np.float32)
    kern = np.asarray(kernel, dtype=np.float32)

    in_maps = prepare_in_maps(x, kern)
    if _program_cache is None:
        _program_cache = _build_program()
    nc = _program_cache

    res = run_bass_kernel_spmd(nc, in_maps, core_ids=list(range(N_CORES)))
    out = np.concatenate([r["out"] for r in res.results], axis=0)
    return out.reshape(B, C, H, W).astype(np.float32)


# revision 34
# speedup vs baseline: 2.3032x; 1.0123x over previous
"""BatchBlur: depthwise 15x15 conv with per-sample kernels, reflection pad 7.

x: (32, 3, 512, 512) f32, kernel: (32, 15, 15) f32 -> out (32, 3, 512, 512) f32.

Strategy: pure data parallel over batch, 4 samples per core on 8 cores.
Host: reflection-pad x to (., 526, 526), cast to fp16, and build dual-band
matrices A[s, k, j, m]: for k<64, A = kern[s, k-m, 2j]; for k>=64,
A = kern[s, k-64-m, 2j+1] (band condition 0 <= dy < 15).
Device: each rhs tile holds the strip rows TWICE — partitions 0:64 at
column offset 0 and partitions 64:128 at column offset 1 (two DMAs straight
from DRAM) — so a single accumulating matmul covers TWO horizontal taps
(dx=2j in the lower band, dx=2j+1 in the upper band) for 50 output rows:
  out[m, n] += sum_k A[k, j, m] * rhs[k, n + 2j]
Eight streams (j=0..7; j=7 contracts only the lower 64 partitions) replace
the fifteen per-tap matmuls. Two images alternate in the two 64-column
halves of the PE array via tile_position=(0,0)/(0,64). fp16 operands with
fp32 PSUM accumulation; eviction casts to fp16 and stores fp16 (halving
store bytes), the host casts back to f32 (~6e-4 relative error total).
Strips: output rows 0..499 in ten 50-row strips plus one final strip at
r0=462 (rows 462..525 = the exact end of the padded image) whose store is
sliced to rows 500..511.
"""
import os
import sys

for _p in ("/opt/trn_rl_repo", "/root/.axon_site/_ro/trn_rl_repo"):
    if _p not in sys.path and os.path.isdir(_p):
        sys.path.insert(0, _p)

import numpy as np

import concourse.bass as bass
import concourse.mybir as mybir
import concourse.tile as tile
from concourse import bacc
from concourse.bass_utils import run_bass_kernel_spmd

L = 15           # blur kernel size
P = L // 2       # reflection pad
B, C, H, W = 32, 3, 512, 512
N_CORES = 8
BS = B // N_CORES            # samples per core
NIMG = BS * C                # channel images per core
HP, WP = H + 2 * P, W + 2 * P  # 526
WPH = WP + 2     # host row pitch: +2 zero cols so the +1-shifted upper band
                 # reads defined data at its last column (no memsets needed)
M_STRIP = 50                 # output rows per strip (dual-band: 2*(50+14)=128)
K_GRP = M_STRIP + L - 1      # 64 input rows per band group
N_DX = (L + 1) // 2          # 8 streams (two taps each; last is single)
R0S = [50 * s for s in range(10)] + [HP - K_GRP]  # last strip 462..525 exactly
N_WARMUP = 100               # dummy matmuls to release the HAM clock gate

F16 = mybir.dt.float16
F32 = mybir.dt.float32

_program_cache = None


def _build_program():
    nc = bacc.Bacc("TRN2", target_bir_lowering=False, debug=False)
    xp_d = nc.dram_tensor("xp", [NIMG, HP, WPH], F16,
                          kind="ExternalInput").ap()
    a_d = nc.dram_tensor("a", [BS, 128, N_DX, M_STRIP], F16,
                         kind="ExternalInput").ap()
    out_d = nc.dram_tensor("out", [NIMG, H, W], F16, kind="ExternalOutput").ap()

    def load_strip(t, img, r0):
        # lower band: rows at column offset 0 (Sync queue); upper band: same
        # rows at column offset 1 (GpSimd queue) => one matmul covers two
        # horizontal taps. The +1 band's last column reads the host's zero
        # padding, so everything is defined without memsets.
        nc.sync.dma_start(out=t[0:K_GRP, 0:WP],
                          in_=xp_d[img, r0:r0 + K_GRP, 0:WP])
        nc.gpsimd.dma_start(out=t[K_GRP:2 * K_GRP, 0:WP],
                            in_=xp_d[img, r0:r0 + K_GRP, 1:WP + 1])

    def load_strip2(t, img, r0):
        # double-strip load: one DMA per band group brings rows for strips
        # r0 and r0+50 (free-dim blocks 0:WP and WP:2*WP). The DRAM source
        # is an overlapping strided view (row stride WPH, strip stride
        # 50*WPH) — plain byte streams, legal for reads.
        base = (img * HP + r0) * WPH
        nc.sync.dma_start(
            out=t[0:K_GRP, :].rearrange("p (q c) -> p q c", c=WP),
            in_=bass.AP(xp_d.tensor, base,
                        [[WPH, K_GRP], [50 * WPH, 2], [1, WP]]))
        nc.gpsimd.dma_start(
            out=t[K_GRP:2 * K_GRP, :].rearrange("p (q c) -> p q c", c=WP),
            in_=bass.AP(xp_d.tensor, base + 1,
                        [[WPH, K_GRP], [50 * WPH, 2], [1, WP]]))

    with tile.TileContext(nc) as tc:
        with (
            tc.tile_pool(name="aconst", bufs=1) as apool,
            tc.tile_pool(name="warm", bufs=1) as wpool,
            tc.tile_pool(name="xin", bufs=6) as xpool,
            tc.tile_pool(name="oout", bufs=4) as opool,
            tc.tile_pool(name="psum", bufs=6, space="PSUM") as psum,
            tc.tile_pool(name="psumw", bufs=1, space="PSUM") as psumw,
        ):
            # HAM warm-up: a burst of full-array matmuls on a zeroed scratch
            # tile releases the PE clock gate (col-tiled matmuls are invisible
            # to the HAM) while the first input DMAs are in flight.
            wsrc = wpool.tile([128, 64], mybir.dt.bfloat16)
            nc.gpsimd.memset(wsrc[:], 0.0)
            wacc = psumw.tile([64, 64], F32)
            for _ in range(N_WARMUP):
                nc.tensor.matmul(wacc[:], wsrc[:, :64], wsrc[:], start=True,
                                 stop=True)

            # first double-strip's image rows: issued before the A load so
            # the DMA queues deliver the first matmuls' dependencies earliest
            xp_first = []
            for img in range(2):
                t = xpool.tile([128, 2 * WP], F16, tag="xp2", name=f"xpf{img}")
                load_strip2(t, img, 0)
                xp_first.append(t)

            # per-sample dual-band matrices: separate tiles => separate
            # dependency tracking; later samples load lazily
            a_t = [
                apool.tile([128, N_DX, M_STRIP], F16, tag=f"a{s}",
                           name=f"a{s}")
                for s in range(BS)
            ]
            nc.sync.dma_start(out=a_t[0][:], in_=a_d[0])

            a_loaded = 0
            for pair in range(NIMG // 2):
                img_a, img_b = 2 * pair, 2 * pair + 1
                smp_a, smp_b = img_a // C, img_b // C
                for s_need in ((2 * pair + 2) // C, (2 * pair + 3) // C):
                    if s_need < BS and s_need > a_loaded:
                        nc.sync.dma_start(out=a_t[s_need][:], in_=a_d[s_need])
                        a_loaded = s_need

                # five double-strip units (rows 0..499) + one single overlap
                # strip at r0=462 storing rows 500..511
                for du in range(6):
                    if du < 5:
                        r0 = 100 * du
                        if pair == 0 and du == 0:
                            xa, xb = xp_first
                        else:
                            xa = xpool.tile([128, 2 * WP], F16, tag="xp2",
                                            name="xa")
                            load_strip2(xa, img_a, r0)
                            xb = xpool.tile([128, 2 * WP], F16, tag="xp2",
                                            name="xb")
                            load_strip2(xb, img_b, r0)
                        # evict as fp16 (DVE casts f32 PSUM) — halves store
                        # bytes; the host casts back to f32 (~2e-4 rel err)
                        o_t = opool.tile([128, 2 * W], F16)
                        for sub in range(2):
                            base = sub * WP
                            acc = psum.tile([128, W], F32)
                            # all 8 streams use K=128 (j=7's upper band is
                            # zero weights) — a K=64 stream would switch the
                            # PE tiling mode and pay a drain twice per strip
                            for j in range(N_DX):
                                nc.tensor.matmul(
                                    acc[0:M_STRIP],
                                    a_t[smp_a][:, j, :],
                                    xa[:, base + 2 * j:base + 2 * j + W],
                                    start=(j == 0),
                                    stop=(j == N_DX - 1),
                                    tile_position=(0, 0),
                                )
                                nc.tensor.matmul(
                                    acc[64:64 + M_STRIP],
                                    a_t[smp_b][:, j, :],
                                    xb[:, base + 2 * j:base + 2 * j + W],
                                    start=(j == 0),
                                    stop=(j == N_DX - 1),
                                    tile_position=(0, 64),
                                )
                            nc.vector.tensor_copy(
                                out=o_t[:, sub * W:(sub + 1) * W],
                                in_=acc[:])
                        # one store per image covers both strips (100
                        # contiguous output rows; non-overlapping views)
                        dva = out_d[img_a, r0:r0 + 2 * M_STRIP, :].rearrange(
                            "(q p) c -> p q c", q=2)
                        dvb = out_d[img_b, r0:r0 + 2 * M_STRIP, :].rearrange(
                            "(q p) c -> p q c", q=2)
                        sva = o_t[0:M_STRIP, :].rearrange(
                            "p (q c) -> p q c", c=W)
                        svb = o_t[64:64 + M_STRIP, :].rearrange(
                            "p (q c) -> p q c", c=W)
                        nc.scalar.dma_start(out=dva, in_=sva)
                        nc.scalar.dma_start(out=dvb, in_=svb)
                    else:
                        r0 = R0S[-1]  # 462
                        lo = 10 * M_STRIP - r0  # store rows 500..511 only
                        xa = xpool.tile([128, WP], F16, tag="xp_t", name="xa1")
                        load_strip(xa, img_a, r0)
                        xb = xpool.tile([128, WP], F16, tag="xp_t", name="xb1")
                        load_strip(xb, img_b, r0)
                        acc = psum.tile([128, W], F32)
                        for j in range(N_DX):
                            nc.tensor.matmul(
                                acc[0:M_STRIP], a_t[smp_a][:, j, :],
                                xa[:, 2 * j:2 * j + W], start=(j == 0),
                                stop=(j == N_DX - 1), tile_position=(0, 0))
                            nc.tensor.matmul(
                                acc[64:64 + M_STRIP], a_t[smp_b][:, j, :],
                                xb[:, 2 * j:2 * j + W], start=(j == 0),
                                stop=(j == N_DX - 1), tile_position=(0, 64))
                        o_s = opool.tile([128, W], F16, tag="o1", name="o1")
                        nc.vector.tensor_copy(out=o_s[:], in_=acc[:])
                        nc.scalar.dma_start(
                            out=out_d[img_a, r0 + lo:r0 + M_STRIP, :],
                            in_=o_s[lo:M_STRIP])
                        nc.scalar.dma_start(
                            out=out_d[img_b, r0 + lo:r0 + M_STRIP, :],
                            in_=o_s[64 + lo:64 + M_STRIP])
    nc.compile()
    return nc


def prepare_in_maps(x: np.ndarray, kern: np.ndarray) -> list:
    # host-side reflection pad, fp16, rows padded to WPH with zero columns
    xpc = np.pad(x, ((0, 0), (0, 0), (P, P), (P, P)), mode="reflect")
    xp = np.zeros((B * C, HP, WPH), dtype=np.float16)
    xp[:, :, :WP] = xpc.reshape(B * C, HP, WP).astype(np.float16)

    # dual-band matrices: lower band = even taps, upper band = odd taps
    kern16 = kern.astype(np.float16)
    a_all = np.zeros((B, 128, N_DX, M_STRIP), dtype=np.float16)
    m_idx = np.arange(M_STRIP)
    for dy in range(L):
        a_all[:, m_idx + dy, :, m_idx] = kern16[:, dy, 0::2]
        a_all[:, K_GRP + m_idx + dy, :L // 2, m_idx] = kern16[:, dy, 1::2]

    return [
        {
            "xp": xp[c * NIMG:(c + 1) * NIMG],
            "a": a_all[c * BS:(c + 1) * BS],
        }
        for c in range(N_CORES)
    ]


def kernel(x: np.ndarray, kernel: np.ndarray) -> np.ndarray:
    global _program_cache
    x = np.asarray(x, dtype=np.float32)
    kern = np.asarray(kernel, dtype=np.float32)

    in_maps = prepare_in_maps(x, kern)
    if _program_cache is None:
        _program_cache = _build_program()
    nc = _program_cache

    res = run_bass_kernel_spmd(nc, in_maps, core_ids=list(range(N_CORES)))
    out = np.concatenate([r["out"] for r in res.results], axis=0)
    return out.reshape(B, C, H, W).astype(np.float32)


# revision 36
# speedup vs baseline: 2.3080x; 1.0021x over previous
"""BatchBlur: depthwise 15x15 conv with per-sample kernels, reflection pad 7.

x: (32, 3, 512, 512) f32, kernel: (32, 15, 15) f32 -> out (32, 3, 512, 512) f32.

Strategy: pure data parallel over batch, 4 samples per core on 8 cores.
Host: reflection-pad x to (., 526, 526), cast to fp16, and build dual-band
matrices A[s, k, j, m]: for k<64, A = kern[s, k-m, 2j]; for k>=64,
A = kern[s, k-64-m, 2j+1] (band condition 0 <= dy < 15).
Device: each rhs tile holds the strip rows TWICE — partitions 0:64 at
column offset 0 and partitions 64:128 at column offset 1 (two DMAs straight
from DRAM) — so a single accumulating matmul covers TWO horizontal taps
(dx=2j in the lower band, dx=2j+1 in the upper band) for 50 output rows:
  out[m, n] += sum_k A[k, j, m] * rhs[k, n + 2j]
Eight streams (j=0..7; j=7 contracts only the lower 64 partitions) replace
the fifteen per-tap matmuls. Two images alternate in the two 64-column
halves of the PE array via tile_position=(0,0)/(0,64). fp16 operands with
fp32 PSUM accumulation; eviction casts to fp16 and stores fp16 (halving
store bytes), the host casts back to f32 (~6e-4 relative error total).
Strips: output rows 0..499 in ten 50-row strips plus one final strip at
r0=462 (rows 462..525 = the exact end of the padded image) whose store is
sliced to rows 500..511.
"""
import os
import sys

for _p in ("/opt/trn_rl_repo", "/root/.axon_site/_ro/trn_rl_repo"):
    if _p not in sys.path and os.path.isdir(_p):
        sys.path.insert(0, _p)

import numpy as np

import concourse.bass as bass
import concourse.mybir as mybir
import concourse.tile as tile
from concourse import bacc
from concourse.bass_utils import run_bass_kernel_spmd

L = 15           # blur kernel size
P = L // 2       # reflection pad
B, C, H, W = 32, 3, 512, 512
N_CORES = 8
BS = B // N_CORES            # samples per core
NIMG = BS * C                # channel images per core
HP, WP = H + 2 * P, W + 2 * P  # 526
WPH = WP + 2     # host row pitch: +2 zero cols so the +1-shifted upper band
                 # reads defined data at its last column (no memsets needed)
M_STRIP = 50                 # output rows per strip (dual-band: 2*(50+14)=128)
K_GRP = M_STRIP + L - 1      # 64 input rows per band group
N_DX = (L + 1) // 2          # 8 streams (two taps each; last is single)
R0S = [50 * s for s in range(10)] + [HP - K_GRP]  # last strip 462..525 exactly
N_WARMUP = 70                # dummy matmuls: release the HAM clock gate and
                             # span the ~3us p-state ramp window

F16 = mybir.dt.float16
F32 = mybir.dt.float32

_program_cache = None


def _build_program():
    nc = bacc.Bacc("TRN2", target_bir_lowering=False, debug=False)
    xp_d = nc.dram_tensor("xp", [NIMG, HP, WPH], F16,
                          kind="ExternalInput").ap()
    a_d = nc.dram_tensor("a", [BS, 128, N_DX, M_STRIP], F16,
                         kind="ExternalInput").ap()
    out_d = nc.dram_tensor("out", [NIMG, H, W], F16, kind="ExternalOutput").ap()

    def load_strip(t, img, r0):
        # lower band: rows at column offset 0 (Sync queue); upper band: same
        # rows at column offset 1 (GpSimd queue) => one matmul covers two
        # horizontal taps. The +1 band's last column reads the host's zero
        # padding, so everything is defined without memsets.
        nc.sync.dma_start(out=t[0:K_GRP, 0:WP],
                          in_=xp_d[img, r0:r0 + K_GRP, 0:WP])
        nc.gpsimd.dma_start(out=t[K_GRP:2 * K_GRP, 0:WP],
                            in_=xp_d[img, r0:r0 + K_GRP, 1:WP + 1])

    def load_strip2(t, img, r0):
        # double-strip load: one DMA per band group brings rows for strips
        # r0 and r0+50 (free-dim blocks 0:WP and WP:2*WP). The DRAM source
        # is an overlapping strided view (row stride WPH, strip stride
        # 50*WPH) — plain byte streams, legal for reads.
        base = (img * HP + r0) * WPH
        nc.sync.dma_start(
            out=t[0:K_GRP, :].rearrange("p (q c) -> p q c", c=WP),
            in_=bass.AP(xp_d.tensor, base,
                        [[WPH, K_GRP], [50 * WPH, 2], [1, WP]]))
        nc.gpsimd.dma_start(
            out=t[K_GRP:2 * K_GRP, :].rearrange("p (q c) -> p q c", c=WP),
            in_=bass.AP(xp_d.tensor, base + 1,
                        [[WPH, K_GRP], [50 * WPH, 2], [1, WP]]))

    with tile.TileContext(nc) as tc:
        with (
            tc.tile_pool(name="aconst", bufs=1) as apool,
            tc.tile_pool(name="warm", bufs=1) as wpool,
            tc.tile_pool(name="xin", bufs=6) as xpool,
            tc.tile_pool(name="oout", bufs=4) as opool,
            tc.tile_pool(name="psum", bufs=6, space="PSUM") as psum,
            tc.tile_pool(name="psumw", bufs=1, space="PSUM") as psumw,
        ):
            # HAM warm-up: a burst of full-array matmuls on a zeroed scratch
            # tile releases the PE clock gate (col-tiled matmuls are invisible
            # to the HAM) while the first input DMAs are in flight.
            # memset on the Vector engine: it dispatches in ~0.3us where the
            # GpSimd Q7 path takes ~3.5us, so the warm-up (and therefore the
            # first real matmul) starts that much earlier
            wsrc = wpool.tile([128, 64], mybir.dt.bfloat16)
            nc.vector.memset(wsrc[:], 0.0)
            wacc = psumw.tile([64, 64], F32)
            for _ in range(N_WARMUP):
                nc.tensor.matmul(wacc[:], wsrc[:, :64], wsrc[:], start=True,
                                 stop=True)

            # first double-strip's image rows: issued before the A load so
            # the DMA queues deliver the first matmuls' dependencies earliest
            xp_first = []
            for img in range(2):
                t = xpool.tile([128, 2 * WP], F16, tag="xp2", name=f"xpf{img}")
                load_strip2(t, img, 0)
                xp_first.append(t)

            # per-sample dual-band matrices: separate tiles => separate
            # dependency tracking; later samples load lazily
            a_t = [
                apool.tile([128, N_DX, M_STRIP], F16, tag=f"a{s}",
                           name=f"a{s}")
                for s in range(BS)
            ]
            nc.sync.dma_start(out=a_t[0][:], in_=a_d[0])

            a_loaded = 0
            for pair in range(NIMG // 2):
                img_a, img_b = 2 * pair, 2 * pair + 1
                smp_a, smp_b = img_a // C, img_b // C
                for s_need in ((2 * pair + 2) // C, (2 * pair + 3) // C):
                    if s_need < BS and s_need > a_loaded:
                        nc.sync.dma_start(out=a_t[s_need][:], in_=a_d[s_need])
                        a_loaded = s_need

                # five double-strip units (rows 0..499) + one single overlap
                # strip at r0=462 storing rows 500..511
                for du in range(6):
                    if du < 5:
                        r0 = 100 * du
                        if pair == 0 and du == 0:
                            xa, xb = xp_first
                        else:
                            xa = xpool.tile([128, 2 * WP], F16, tag="xp2",
                                            name="xa")
                            load_strip2(xa, img_a, r0)
                            xb = xpool.tile([128, 2 * WP], F16, tag="xp2",
                                            name="xb")
                            load_strip2(xb, img_b, r0)
                        # evict as fp16 (DVE casts f32 PSUM) — halves store
                        # bytes; the host casts back to f32 (~2e-4 rel err)
                        o_t = opool.tile([128, 2 * W], F16)
                        for sub in range(2):
                            base = sub * WP
                            acc = psum.tile([128, W], F32)
                            # all 8 streams use K=128 (j=7's upper band is
                            # zero weights) — a K=64 stream would switch the
                            # PE tiling mode and pay a drain twice per strip
                            for j in range(N_DX):
                                nc.tensor.matmul(
                                    acc[0:M_STRIP],
                                    a_t[smp_a][:, j, :],
                                    xa[:, base + 2 * j:base + 2 * j + W],
                                    start=(j == 0),
                                    stop=(j == N_DX - 1),
                                    tile_position=(0, 0),
                                )
                                nc.tensor.matmul(
                                    acc[64:64 + M_STRIP],
                                    a_t[smp_b][:, j, :],
                                    xb[:, base + 2 * j:base + 2 * j + W],
                                    start=(j == 0),
                                    stop=(j == N_DX - 1),
                                    tile_position=(0, 64),
                                )
                            nc.vector.tensor_copy(
                                out=o_t[:, sub * W:(sub + 1) * W],
                                in_=acc[:])
                        # one store per image covers both strips (100
                        # contiguous output rows; non-overlapping views)
                        dva = out_d[img_a, r0:r0 + 2 * M_STRIP, :].rearrange(
                            "(q p) c -> p q c", q=2)
                        dvb = out_d[img_b, r0:r0 + 2 * M_STRIP, :].rearrange(
                            "(q p) c -> p q c", q=2)
                        sva = o_t[0:M_STRIP, :].rearrange(
                            "p (q c) -> p q c", c=W)
                        svb = o_t[64:64 + M_STRIP, :].rearrange(
                            "p (q c) -> p q c", c=W)
                        nc.scalar.dma_start(out=dva, in_=sva)
                        nc.scalar.dma_start(out=dvb, in_=svb)
                    else:
                        r0 = R0S[-1]  # 462
                        lo = 10 * M_STRIP - r0  # store rows 500..511 only
                        xa = xpool.tile([128, WP], F16, tag="xp_t", name="xa1")
                        load_strip(xa, img_a, r0)
                        xb = xpool.tile([128, WP], F16, tag="xp_t", name="xb1")
                        load_strip(xb, img_b, r0)
                        acc = psum.tile([128, W], F32)
                        for j in range(N_DX):
                            nc.tensor.matmul(
                                acc[0:M_STRIP], a_t[smp_a][:, j, :],
                                xa[:, 2 * j:2 * j + W], start=(j == 0),
                                stop=(j == N_DX - 1), tile_position=(0, 0))
                            nc.tensor.matmul(
                                acc[64:64 + M_STRIP], a_t[smp_b][:, j, :],
                                xb[:, 2 * j:2 * j + W], start=(j == 0),
                                stop=(j == N_DX - 1), tile_position=(0, 64))
                        o_s = opool.tile([128, W], F16, tag="o1", name="o1")
                        nc.vector.tensor_copy(out=o_s[:], in_=acc[:])
                        nc.scalar.dma_start(
                            out=out_d[img_a, r0 + lo:r0 + M_STRIP, :],
                            in_=o_s[lo:M_STRIP])
                        nc.scalar.dma_start(
                            out=out_d[img_b, r0 + lo:r0 + M_STRIP, :],
                            in_=o_s[64 + lo:64 + M_STRIP])
    nc.compile()
    return nc


def prepare_in_maps(x: np.ndarray, kern: np.ndarray) -> list:
    # host-side reflection pad, fp16, rows padded to WPH with zero columns
    xpc = np.pad(x, ((0, 0), (0, 0), (P, P), (P, P)), mode="reflect")
    xp = np.zeros((B * C, HP, WPH), dtype=np.float16)
    xp[:, :, :WP] = xpc.reshape(B * C, HP, WP).astype(np.float16)

    # dual-band matrices: lower band = even taps, upper band = odd taps
    kern16 = kern.astype(np.float16)
    a_all = np.zeros((B, 128, N_DX, M_STRIP), dtype=np.float16)
    m_idx = np.arange(M_STRIP)
    for dy in range(L):
        a_all[:, m_idx + dy, :, m_idx] = kern16[:, dy, 0::2]
        a_all[:, K_GRP + m_idx + dy, :L // 2, m_idx] = kern16[:, dy, 1::2]

    return [
        {
            "xp": xp[c * NIMG:(c + 1) * NIMG],
            "a": a_all[c * BS:(c + 1) * BS],
        }
        for c in range(N_CORES)
    ]


def kernel(x: np.ndarray, kernel: np.ndarray) -> np.ndarray:
    global _program_cache
    x = np.asarray(x, dtype=np.float32)
    kern = np.asarray(kernel, dtype=np.float32)

    in_maps = prepare_in_maps(x, kern)
    if _program_cache is None:
        _program_cache = _build_program()
    nc = _program_cache

    res = run_bass_kernel_spmd(nc, in_maps, core_ids=list(range(N_CORES)))
    out = np.concatenate([r["out"] for r in res.results], axis=0)
    return out.reshape(B, C, H, W).astype(np.float32)
